# revision 1
# baseline (speedup 1.0000x reference)
"""Trainium2 Bass kernel for nn_ArnDiffRnnAgent (dense_mlp, 8-core data parallel).

Strategy
--------
Pure data parallel: batch 8192 is split into 8 shards of 1024 rows, one per
NeuronCore; the (small) weights are replicated.  All activations live in SBUF
feature-major ("transposed": [feature, batch]) so every GEMM maps directly to
the TensorEngine (contraction dim on partitions).  Compute dtype is bf16 with
fp32 PSUM accumulation; inputs are cast f32->bf16 by SWDGE cast-DMAs and
transposed via the DMA XBAR (bf16-only), outputs are staged bf16 feature-major
in DRAM, DMA-transposed back to batch-major and cast-stored as f32.

Host-side weight prep (legit parameter layout prep, amortized in any real use):
 - all weights pre-transposed/tiled to [128, k_tiles, M] bf16
 - enemy e2-layer folded into the GRU input weights:
      gi = e2 @ wih.T + bih,  e2 = relu1 @ W_e2.T + b_e2
   => gi = relu1 @ (wih @ W_e2).T + (wih @ b_e2 + bih)
 - r/z gates of each GRU computed as one K=512 GEMM over [x_side | h_side]
"""
import sys
sys.path.insert(0, "/opt/trn_rl_repo")

import numpy as np
import ml_dtypes

import concourse.bass as bass  # noqa: F401
import concourse.tile as tile
from concourse import bacc, mybir
from concourse.bass_utils import run_bass_kernel_spmd

BF16 = mybir.dt.bfloat16
F32 = mybir.dt.float32
AF = mybir.ActivationFunctionType
ALU = mybir.AluOpType
bf16 = ml_dtypes.bfloat16

N_CORES = 8
B = 8192
IN = 1504
INP = 1536          # padded to 12*128
H = 256
E = 64
MOVE = 16
N_EN = 16
HID = (1 + N_EN) * H  # 4352

_CACHE = {}


# --------------------------------------------------------------------------
# host-side weight prep
# --------------------------------------------------------------------------
def _ktile(wt, m=None):
    """[K, M] (f32) -> [128, K//128, M] bf16 host array (K padded to 128s)."""
    K = wt.shape[0]
    Kp = ((K + 127) // 128) * 128
    if Kp != K:
        wt = np.concatenate([wt, np.zeros((Kp - K, wt.shape[1]), np.float32)], 0)
    return np.ascontiguousarray(
        wt.reshape(Kp // 128, 128, wt.shape[1]).transpose(1, 0, 2)
    ).astype(bf16)


def _btile(b):
    """[M] f32 -> [128, M//128] f32."""
    return np.ascontiguousarray(b.reshape(-1, 128).T).astype(np.float32)


def prep_weights(i):
    w = {}
    f32 = np.float32
    w["wenv1"] = _ktile(i["W_env1"].T)                      # [128,12,256]
    w["benv1"] = _btile(i["b_env1"])
    w["wenv2"] = _ktile(i["W_env2"].T)                      # [128,2,256]
    w["benv2"] = _btile(i["b_env2"])

    def gru(tag, wih, whh, bih, bhh):
        w["wrz" + tag] = _ktile(
            np.concatenate([wih[:512].T, whh[:512].T], 0))   # [128,4,512]
        w["brz" + tag] = _btile(bih[:512] + bhh[:512])
        w["wgin" + tag] = _ktile(wih[512:].T)                # [128,2,256]
        w["bgin" + tag] = _btile(bih[512:])
        w["wghn" + tag] = _ktile(whh[512:].T)
        w["bghn" + tag] = _btile(bhh[512:])

    gru("e", i["wih_env"].astype(f32), i["whh_env"].astype(f32),
        i["bih_env"].astype(f32), i["bhh_env"].astype(f32))

    for tag in ("a", "b"):
        wih = i["wih_e" + tag].astype(f32)
        whh = i["whh_e" + tag].astype(f32)
        bih = i["bih_e" + tag].astype(f32)
        bhh = i["bhh_e" + tag].astype(f32)
        We2 = i["W_e2" + tag].astype(f32)
        be2 = i["b_e2" + tag].astype(f32)
        Wgi = wih @ We2                                       # [768,256]
        bgi = wih @ be2 + bih
        gru(tag, Wgi, whh, bgi, bhh)
        # e1 weights duplicated in both partition halves (odd enemies sit at
        # base_partition 64 and matmul requires equal operand bases)
        e1t = i["W_e1" + tag].astype(f32).T                   # [64,256]
        w["we1" + tag] = np.ascontiguousarray(
            np.concatenate([e1t, e1t], 0)).astype(bf16)       # [128,256]
        w["be1" + tag] = _btile(i["b_e1" + tag].astype(f32))

    w["wwo"] = _ktile(i["W_wo"].T)                            # [128,2,6]
    w["bwo"] = np.ascontiguousarray(
        i["b_wo"].astype(f32).reshape(6, 1))
    sel = np.zeros((128, 64), np.float32)
    for j in range(8):
        sel[:, j * 8 + j] = 1.0
    w["sel"] = sel.astype(bf16)
    return w


WEIGHT_SPECS = {
    "wenv1": ([128, 12, 256], BF16), "benv1": ([128, 2], F32),
    "wenv2": ([128, 2, 256], BF16), "benv2": ([128, 2], F32),
    "wwo": ([128, 2, 6], BF16), "bwo": ([6, 1], F32),
    "sel": ([128, 64], BF16),
}
for _t in ("e", "a", "b"):
    WEIGHT_SPECS["wrz" + _t] = ([128, 4, 512], BF16)
    WEIGHT_SPECS["brz" + _t] = ([128, 4], F32)
    WEIGHT_SPECS["wgin" + _t] = ([128, 2, 256], BF16)
    WEIGHT_SPECS["bgin" + _t] = ([128, 2], F32)
    WEIGHT_SPECS["wghn" + _t] = ([128, 2, 256], BF16)
    WEIGHT_SPECS["bghn" + _t] = ([128, 2], F32)
for _t in ("a", "b"):
    WEIGHT_SPECS["we1" + _t] = ([128, 256], BF16)
    WEIGHT_SPECS["be1" + _t] = ([128, 2], F32)


# --------------------------------------------------------------------------
# device kernel builder
# --------------------------------------------------------------------------
def build(bc=1024, chunk=512):
    """bc = batch rows per core, chunk = moving free-dim per GEMM."""
    nc = bacc.Bacc("TRN2", target_bir_lowering=False, debug=False,
                   num_devices=N_CORES)

    x = nc.dram_tensor("x", [bc, IN], F32, kind="ExternalInput")
    h = nc.dram_tensor("h", [bc, HID], F32, kind="ExternalInput")
    W = {k: nc.dram_tensor(k, s, d, kind="ExternalInput")
         for k, (s, d) in WEIGHT_SPECS.items()}

    q_out = nc.dram_tensor("q", [bc, 22], F32, kind="ExternalOutput")
    h_out = nc.dram_tensor("hidden", [bc, HID], F32, kind="ExternalOutput")

    x16f = nc.dram_tensor("x16f", [bc, INP], BF16)
    x16e = nc.dram_tensor("x16e", [bc, N_EN * E], BF16)
    h16 = nc.dram_tensor("h16", [bc, HID], BF16)
    hoT = nc.dram_tensor("hoT", [HID, bc], BF16)
    qT16 = nc.dram_tensor("qT16", [32, bc], BF16)

    nch = bc // chunk

    with tile.TileContext(nc) as tc:
        with (
            tc.tile_pool(name="wts", bufs=1) as wpool,
            tc.tile_pool(name="xt", bufs=2) as xt_pool,
            tc.tile_pool(name="ft", bufs=2) as ft_pool,
            tc.tile_pool(name="ht", bufs=6) as ht_pool,
            tc.tile_pool(name="ev", bufs=3) as ev_pool,
            tc.tile_pool(name="am", bufs=18) as am_pool,
            tc.tile_pool(name="outp", bufs=2) as out_pool,
            tc.tile_pool(name="psA", bufs=2, space="PSUM") as psA,
            tc.tile_pool(name="psB", bufs=4, space="PSUM") as psB,
        ):
            # ---- staging casts / copies --------------------------------
            nc.gpsimd.dma_start(out=x16f[:, 0:IN], in_=x[:, :])
            zt = wpool.tile([128, (bc * (INP - IN)) // 128], BF16, name="zt")
            nc.gpsimd.memset(zt[:], 0.0)
            nc.sync.dma_start(out=x16f[:, IN:INP], in_=zt[:])
            nc.sync.dma_start(out=x16e[:, :], in_=x16f[:, MOVE:MOVE + N_EN * E])
            nc.gpsimd.dma_start(out=h16[:, :], in_=h[:, :])

            # ---- weights into SBUF -------------------------------------
            wt = {}
            for k, (s, d) in WEIGHT_SPECS.items():
                wt[k] = wpool.tile(s, d, name="w_" + k)
                nc.sync.dma_start(wt[k][:], W[k][:])

            def mm(psum_ap, lhs_ap, mov_ap, start, stop):
                nc.tensor.matmul(psum_ap, lhs_ap, mov_ap, start=start, stop=stop)

            # full GEMM helper: out rows = M (tiles of 128), moving k-tiles
            def gemm(ps_tile, wkey, movs, m_tiles, n, extra=None):
                """ps_tile: psum tile [128, m_tiles, n]; movs: list of (tile, kslice)
                APs [128, n] supplying contraction tiles in order."""
                wtile = wt[wkey]
                nk = len(movs)
                for m in range(m_tiles):
                    for ki, mov in enumerate(movs):
                        mm(ps_tile[:, m, :],
                           wtile[:, ki, 128 * m:128 * (m + 1)],
                           mov, ki == 0, ki == nk - 1)

            # ---- main loop ---------------------------------------------
            for c in range(nch):
                b0 = c * chunk

                # transposed env input: [128, 12, chunk]
                xt = xt_pool.tile([128, INP // 128, chunk], BF16, name="xt")
                for k in range(INP // 128):
                    nc.sync.dma_start(
                        out=xt[:, k, :],
                        in_=x16f[b0:b0 + chunk, 128 * k:128 * (k + 1)],
                        transpose=True)

                # enemy features: 8 tiles of [128, chunk] covering 2 enemies each
                ft = ft_pool.tile([128, 8, chunk], BF16, name="ft")
                for j in range(8):
                    nc.sync.dma_start(
                        out=ft[:, j, :],
                        in_=x16e[b0:b0 + chunk, 128 * j:128 * (j + 1)],
                        transpose=True)

                # -------- shared GRU tail -------------------------------
                def gru_tail(tag, rz_ps, gin_ps, ghn_ps, hT, m_tiles=2):
                    """returns h' [128, m_tiles, chunk] bf16 given psums + hT."""
                    rzs = ev_pool.tile([128, 2 * m_tiles, chunk], BF16, name="rzs")
                    for j in range(2 * m_tiles):
                        nc.scalar.activation(
                            rzs[:, j, :], rz_ps[j][:], AF.Sigmoid,
                            bias=wt["brz" + tag][:, j:j + 1])
                    t1 = ev_pool.tile([128, m_tiles, chunk], BF16, name="t1")
                    t2 = ev_pool.tile([128, m_tiles, chunk], BF16, name="t2")
                    nn = ev_pool.tile([128, m_tiles, chunk], BF16, name="nn")
                    hp = ev_pool.tile([128, m_tiles, chunk], BF16, name="hp")
                    for m in range(m_tiles):
                        nc.vector.scalar_tensor_tensor(
                            t1[:, m, :], ghn_ps[:, m, :],
                            wt["bghn" + tag][:, m:m + 1], rzs[:, m, :],
                            op0=ALU.add, op1=ALU.mult)
                        nc.vector.scalar_tensor_tensor(
                            t2[:, m, :], gin_ps[:, m, :],
                            wt["bgin" + tag][:, m:m + 1], t1[:, m, :],
                            op0=ALU.add, op1=ALU.add)
                        nc.scalar.activation(nn[:, m, :], t2[:, m, :], AF.Tanh)
                        # h' = n + z*(h-n)
                        nc.vector.tensor_sub(t1[:, m, :], hT[:, m, :], nn[:, m, :])
                        nc.vector.tensor_mul(t2[:, m, :], rzs[:, m_tiles + m, :],
                                             t1[:, m, :])
                        nc.vector.tensor_add(hp[:, m, :], nn[:, m, :], t2[:, m, :])
                    return hp

                # -------- env pathway -----------------------------------
                pe1 = psA.tile([128, 2, chunk], F32, name="pe1", tag="psA")
                gemm(pe1, "wenv1", [xt[:, k, :] for k in range(INP // 128)], 2, chunk)
                eh1 = ev_pool.tile([128, 2, chunk], BF16, name="eh1")
                for m in range(2):
                    nc.scalar.activation(eh1[:, m, :], pe1[:, m, :], AF.Relu,
                                         bias=wt["benv1"][:, m:m + 1])

                pe2 = psA.tile([128, 2, chunk], F32, name="pe2", tag="psA")
                gemm(pe2, "wenv2", [eh1[:, k, :] for k in range(2)], 2, chunk)
                eh2 = ev_pool.tile([128, 2, chunk], BF16, name="eh2", bufs=2)
                for m in range(2):
                    nc.scalar.activation(eh2[:, m, :], pe2[:, m, :], AF.Identity,
                                         bias=wt["benv2"][:, m:m + 1])

                hTe = ht_pool.tile([128, 2, chunk], BF16, name="hTe")
                for k in range(2):
                    nc.sync.dma_start(out=hTe[:, k, :],
                                      in_=h16[b0:b0 + chunk, 128 * k:128 * (k + 1)],
                                      transpose=True)

                movs_rz = [eh2[:, 0, :], eh2[:, 1, :], hTe[:, 0, :], hTe[:, 1, :]]
                rz_ps = [psB.tile([128, chunk], F32, name="prz%d" % j, tag="psB")
                         for j in range(4)]
                for j in range(4):
                    for ki, mov in enumerate(movs_rz):
                        mm(rz_ps[j][:], wt["wrze"][:, ki, 128 * j:128 * (j + 1)],
                           mov, ki == 0, ki == 3)
                gin = psA.tile([128, 2, chunk], F32, name="gin", tag="psA")
                gemm(gin, "wgine", [eh2[:, k, :] for k in range(2)], 2, chunk)
                ghn = psA.tile([128, 2, chunk], F32, name="ghn", tag="psA")
                gemm(ghn, "wghne", [hTe[:, k, :] for k in range(2)], 2, chunk)

                henv = gru_tail("e", rz_ps, gin, ghn, hTe)
                nc.sync.dma_start(
                    out=hoT[0:H, b0:b0 + chunk].rearrange(
                        "(k p) b -> p k b", p=128),
                    in_=henv[:])

                # wo_q = henv @ W_wo.T + b_wo   -> qT16 rows 0:6
                pwo = psA.tile([6, chunk], F32, name="pwo", tag="psA")
                for ki in range(2):
                    mm(pwo[:], wt["wwo"][:, ki, :], henv[:, ki, :], ki == 0, ki == 1)
                qwo = ev_pool.tile([6, chunk], BF16, name="qwo")
                nc.scalar.activation(qwo[:], pwo[:], AF.Identity, bias=wt["bwo"][:])
                nc.sync.dma_start(out=qT16[0:6, b0:b0 + chunk], in_=qwo[:])

                # -------- enemies ---------------------------------------
                ams = []
                for t in range(N_EN):
                    tag = "a" if t < 8 else "b"
                    half = 64 * (t % 2)
                    fT = ft[half:half + 64, t // 2, :]

                    hTt = ht_pool.tile([128, 2, chunk], BF16, name="hTt")
                    for k in range(2):
                        c0 = H * (1 + t) + 128 * k
                        nc.sync.dma_start(out=hTt[:, k, :],
                                          in_=h16[b0:b0 + chunk, c0:c0 + 128],
                                          transpose=True)

                    ep1 = psA.tile([128, 2, chunk], F32, name="ep1", tag="psA")
                    for m in range(2):
                        mm(ep1[:, m, :],
                           wt["we1" + tag][half:half + 64, 128 * m:128 * (m + 1)],
                           fT, True, True)
                    e1r = ev_pool.tile([128, 2, chunk], BF16, name="e1r")
                    for m in range(2):
                        nc.scalar.activation(e1r[:, m, :], ep1[:, m, :], AF.Relu,
                                             bias=wt["be1" + tag][:, m:m + 1])

                    movs = [e1r[:, 0, :], e1r[:, 1, :], hTt[:, 0, :], hTt[:, 1, :]]
                    rz_ps = [psB.tile([128, chunk], F32, name="erz%d" % j, tag="psB")
                             for j in range(4)]
                    for j in range(4):
                        for ki, mov in enumerate(movs):
                            mm(rz_ps[j][:],
                               wt["wrz" + tag][:, ki, 128 * j:128 * (j + 1)],
                               mov, ki == 0, ki == 3)
                    gin = psA.tile([128, 2, chunk], F32, name="egin", tag="psA")
                    gemm(gin, "wgin" + tag, [e1r[:, k, :] for k in range(2)], 2, chunk)
                    ghn = psA.tile([128, 2, chunk], F32, name="eghn", tag="psA")
                    gemm(ghn, "wghn" + tag, [hTt[:, k, :] for k in range(2)], 2, chunk)

                    h3 = gru_tail(tag, rz_ps, gin, ghn, hTt)
                    nc.sync.dma_start(
                        out=hoT[H * (1 + t):H * (2 + t), b0:b0 + chunk].rearrange(
                            "(k p) b -> p k b", p=128),
                        in_=h3[:])

                    am = am_pool.tile([128, 2, chunk], BF16, name="am")
                    for m in range(2):
                        nc.vector.tensor_mul(am[:, m, :], eh2[:, m, :], h3[:, m, :])
                    ams.append(am)

                # -------- attack reduction (chunk end) ------------------
                for ty in range(2):
                    patt = psA.tile([8, chunk], F32, name="patt", tag="psA")
                    for j in range(8):
                        for k in range(2):
                            mm(patt[:], wt["sel"][:, 8 * j:8 * j + 8],
                               ams[8 * ty + j][:, k, :],
                               j == 0 and k == 0, j == 7 and k == 1)
                    att = ev_pool.tile([8, chunk], BF16, name="att")
                    nc.vector.tensor_copy(att[:], patt[:])
                    nc.sync.dma_start(
                        out=qT16[6 + 8 * ty:14 + 8 * ty, b0:b0 + chunk], in_=att[:])

            # ---- final un-transpose + f32 stores -----------------------
            for bt in range(bc // 128):
                r0 = 128 * bt
                ho = out_pool.tile([128, HID], BF16, name="ho")
                nc.sync.dma_start(out=ho[:], in_=hoT[:, r0:r0 + 128],
                                  transpose=True)
                nc.gpsimd.dma_start(out=h_out[r0:r0 + 128, :], in_=ho[:])

                qs = out_pool.tile([128, 32], BF16, name="qs")
                nc.sync.dma_start(out=qs[:], in_=qT16[:, r0:r0 + 128],
                                  transpose=True)
                nc.gpsimd.dma_start(out=q_out[r0:r0 + 128, :], in_=qs[:, 0:22])

    nc.compile()
    return nc


# --------------------------------------------------------------------------
# public entry point
# --------------------------------------------------------------------------
def _get_nc(bc, chunk):
    key = (bc, chunk)
    if key not in _CACHE:
        _CACHE[key] = build(bc, chunk)
    return _CACHE[key]


def run(inputs, bc=1024, chunk=512, trace=False):
    inputs = {k: np.asarray(v) for k, v in inputs.items()}
    nc = _get_nc(bc, chunk)
    w = prep_weights(inputs)
    x = np.ascontiguousarray(inputs["inputs"], np.float32)
    h = np.ascontiguousarray(inputs["hidden_state"], np.float32)
    n_cores = N_CORES
    assert x.shape[0] == bc * n_cores
    in_maps = []
    for i in range(n_cores):
        m = {"x": np.ascontiguousarray(x[i * bc:(i + 1) * bc]),
             "h": np.ascontiguousarray(h[i * bc:(i + 1) * bc])}
        m.update(w)
        in_maps.append(m)
    res = run_bass_kernel_spmd(nc, in_maps, list(range(n_cores)), trace=trace)
    q = np.concatenate([r["q"] for r in res.results], 0)
    hid = np.concatenate([r["hidden"] for r in res.results], 0)
    return (q, hid), res


def kernel(**inputs):
    (q, hid), _ = run(inputs, bc=B // N_CORES, chunk=512)
    return q, hid
